# revision 9
# baseline (speedup 1.0000x reference)
"""Trainium2 Bass kernel for the CNNFusing ragged-session attention pooling module.

Computes, per session s over its token set:
    v_mean   = mean(hidden[s])                                  [H]
    ph[t]    = tanh(hidden[t] @ Wp1 + (pos_table @ Wp2 + b_pos)[rp[t]])
    gate[t]  = sigmoid(v_mean @ W1 + b1 + ph[t] @ W2 + b2)
    alpha[t] = gate[t] @ qw + qb
    h_s      = sum_t alpha[t] * hidden[t]                       [B, H]

Strategy: sessions are bin-packed (worst-fit decreasing) into 512-token
chunks spread over 8 cores — pure data parallelism. All ragged ops become
one-hot matmuls. The two big matmuls per chunk (ph and gate pre-activations)
run as fp8e4m3 DoubleRow matmuls with *residual pairs*: the moving operand
carries fp8(x) and fp8(x - fp8(x)) k-tile pairs and the stationary weights
carry an fp8 residual pass, recovering ~fp16 accuracy at a quarter of the
fp16 PE cost. The position gather is a host-built one-hot that rides the
same DoubleRow pipe (rp < 128 for this input distribution). h_s and session
sums are computed transposed ([H, S]) so their matmuls move S=32 columns
instead of H=256; the host untransposes. Everything the PE consumes is
pre-laid-out on the host, so no on-device DMA transposes are needed.
"""

import numpy as np
import ml_dtypes

import concourse.bass as bass
import concourse.mybir as mybir
import concourse.tile as tile
from concourse.bass_utils import run_bass_kernel_spmd

H = 256
TC = 512      # tokens per chunk
S = 32        # max sessions per chunk
KT = TC // 128
N_CORES = 8

F8 = mybir.dt.float8e4
F16 = mybir.dt.float16
F32 = mybir.dt.float32
NP8 = ml_dtypes.float8_e4m3
DR = mybir.MatmulPerfMode.DoubleRow


# --------------------------------------------------------------------------
# The walrus build here accepts only ONE sync-wait command per instruction,
# while Tile may attach several. Hoist all but the last wait onto standalone
# event-semaphore waits inserted just before them on the same engine.
_waitsplit_uid = [0]


def _split_multi_waits(nc):
    for fn in nc.m.functions:
        for bb in fn.blocks:
            insts = bb.instructions
            i = 0
            while i < len(insts):
                inst = insts[i]
                si = getattr(inst, "sync_info", None)
                waits = list(si.on_wait) if si is not None and si.on_wait else []
                if len(waits) > 1:
                    si.on_wait = waits[-1:]
                    for w in waits[:-1]:
                        ev = mybir.InstEventSemaphore(
                            name=f"I-waitsplit-{_waitsplit_uid[0]}", ins=[], outs=[]
                        )
                        _waitsplit_uid[0] += 1
                        ev.engine = inst.engine
                        ev.sync_info = mybir.SyncInfo(on_wait=[w], on_update=[])
                        insts.insert(i, ev)
                        i += 1
                i += 1
# --------------------------------------------------------------------------


def _q8(a):
    return np.asarray(a, np.float32).astype(NP8)


def _plan(seq_len):
    """Worst-fit decreasing bin packing of sessions into N_CORES*C bins of
    <= TC tokens and <= S sessions. Returns (C, bin_sessions) where
    bin_sessions[b] is the list of session ids in bin b; bin b belongs to
    core b // C, chunk b % C."""
    lens = np.asarray(seq_len, dtype=np.int64)
    B = len(lens)
    total = int(lens.sum())
    order = np.argsort(-lens, kind="stable")
    C = max(1, -(-total // (N_CORES * TC)))
    while True:
        nb = N_CORES * C
        free = np.full(nb, TC, np.int64)
        cnt = np.zeros(nb, np.int64)
        bins = [[] for _ in range(nb)]
        ok = True
        for sid in order:
            l = lens[sid]
            cand = np.where((free >= l) & (cnt < S))[0]
            if len(cand) == 0:
                ok = False
                break
            b = cand[np.argmax(free[cand])]
            bins[b].append(int(sid))
            free[b] -= l
            cnt[b] += 1
        if ok:
            return C, bins
        C += 1


def _pack_inputs(hidden, reverse_pos, seq_len, C, bins):
    """Build all per-core device arrays."""
    lens = np.asarray(seq_len, dtype=np.int64)
    starts = np.cumsum(lens) - lens
    B = len(lens)
    NB = N_CORES * C

    tok_idx = np.zeros((NB, TC), np.int64)
    valid = np.zeros((NB, TC), bool)
    seg_local = np.full((NB, TC), -1.0, np.float32)
    recip = np.zeros((NB, S), np.float32)
    out_core = np.zeros(B, np.int32)
    out_chunk = np.zeros(B, np.int32)
    out_local = np.zeros(B, np.int32)

    for b, sess in enumerate(bins):
        t = 0
        for j, sid in enumerate(sess):
            l = int(lens[sid])
            tok_idx[b, t : t + l] = np.arange(starts[sid], starts[sid] + l)
            valid[b, t : t + l] = True
            seg_local[b, t : t + l] = j
            recip[b, j] = 1.0 / l
            out_core[sid] = b // C
            out_chunk[sid] = b % C
            out_local[sid] = j
            t += l

    rp = np.asarray(reverse_pos)[tok_idx]
    rp[~valid] = 255  # no one-hot row matches -> zero pos contribution
    assert rp[valid].max() < 128, "reverse_pos >= 128 unsupported by one-hot"

    xg = np.asarray(hidden, np.float32)[tok_idx]
    xg[~valid] = 0.0

    # xm8 [NB, 128, 5, TC]: x0 x1 r0 r1 oh  (h-major transposed, fp8+resid;
    # the (oh, oh) DoubleRow pair is formed on-device with a stride-0 AP)
    xT = np.ascontiguousarray(xg.reshape(NB, TC, 2, 128).transpose(0, 3, 2, 1))
    x8T = xT.astype(NP8)
    r8T = (xT - x8T.astype(np.float32)).astype(NP8)
    oh8 = (rp[:, None, :] == np.arange(128)[None, :, None]).astype(NP8)
    xm8 = np.concatenate(
        [x8T, r8T, oh8[:, :, None, :]], axis=2
    )  # [NB, 128, 5, TC]

    # x16 [NB, 128, KT*H]: token-partition layout for ss / h_sT lhsT
    x16 = np.ascontiguousarray(
        xg.reshape(NB, KT, 128, H).transpose(0, 2, 1, 3)
    ).astype(np.float16).reshape(NB, 128, KT * H)

    # a_s [NB, 32, TC] f16: session one-hot over tokens
    a_s = (
        seg_local[:, None, :] == np.arange(S, dtype=np.float32)[None, :, None]
    ).astype(np.float16)

    # seg_col [N_CORES, 128, C, KT] f32 for on-device a_t masks
    seg_col = np.ascontiguousarray(
        seg_local.reshape(N_CORES, C, KT, 128).transpose(0, 3, 1, 2)
    ).astype(np.float32)

    recip = np.ascontiguousarray(
        recip.reshape(N_CORES, C, S).transpose(0, 2, 1)
    )  # [N_CORES, S, C]

    shp = lambda a: a.reshape((N_CORES, C) + a.shape[1:])
    return (
        shp(xm8.reshape(NB, 128, 5 * TC)),
        shp(x16),
        shp(a_s),
        seg_col,
        recip,
        (out_core, out_chunk, out_local),
    )


def _pack_weights(pos_table, W_pos, b_pos, W1, b1, W2, b2, qw, qb):
    def pairs8(M):
        """[256, H] f32 -> fp8 pair tiles [128, 2, H] plus residual tiles."""
        M = np.asarray(M, np.float32)
        t = np.ascontiguousarray(M.reshape(2, 128, M.shape[1]).transpose(1, 0, 2))
        t8 = t.astype(NP8)
        t8r = (t - t8.astype(np.float32)).astype(NP8)
        return t8, t8r

    Wp = np.asarray(W_pos, np.float32)
    wp18, wp18r = pairs8(Wp[:H])
    pwf = np.asarray(pos_table, np.float32) @ Wp[H:] + np.asarray(b_pos, np.float32)
    pp = np.zeros((128, H), np.float32)
    n = min(128, pwf.shape[0])  # rp < 128 for this input distribution
    pp[:n] = pwf[:n]
    pp8 = pp.astype(NP8)
    pp8r = (pp - pp8.astype(np.float32)).astype(NP8)
    pp8c = np.ascontiguousarray(np.stack([pp8, pp8r], 1))  # [128, 2, H]

    w18, w18r = pairs8(np.asarray(W1, np.float32))
    w28, w28r = pairs8(np.asarray(W2, np.float32))

    qwf = np.asarray(qw, np.float32).reshape(H)
    # alpha = gate@qw + qb with gate = 0.5*gt + 0.5 folds to
    # alpha = 0.5*(gt@qw) + (qb + sum(qw)/2); the 0.5 applied post-matmul.
    qwh = np.ascontiguousarray(qwf.reshape(2, 128).T).astype(np.float16)
    qbp = float(np.asarray(qb, np.float32).reshape(()) + qwf.sum() / 2.0)
    # full b1+b2, folded into g1 (rides the session one-hot broadcast); the
    # gate activation then needs no per-half bias and can be one instruction.
    bcf = np.asarray(b1, np.float32) + np.asarray(b2, np.float32)
    bcg = np.broadcast_to(bcf.astype(np.float16), (S, H)).copy()

    iota_at = np.broadcast_to(np.arange(S, dtype=np.float16), (128, S)).copy()
    return dict(
        wp18=wp18, wp18r=wp18r, pp8c=pp8c, w18=w18, w18r=w18r,
        w28=w28, w28r=w28r, qwh=qwh, bcg=bcg, iota_at=iota_at,
    ), qbp


def _build_bass(C, qbp):
    nc = bass.Bass("TRN2", target_bir_lowering=False, debug=False,
                   num_devices=N_CORES)

    xm8 = nc.dram_tensor("xm8", [C, 128, 5 * TC], F8, kind="ExternalInput")
    x16 = nc.dram_tensor("x16", [C, 128, KT * H], F16, kind="ExternalInput")
    a_s = nc.dram_tensor("a_s", [C, S, TC], F16, kind="ExternalInput")
    segc = nc.dram_tensor("segc", [128, C, KT], F32, kind="ExternalInput")
    recip = nc.dram_tensor("recip", [S, C], F32, kind="ExternalInput")
    wp18 = nc.dram_tensor("wp18", [128, 2, H], F8, kind="ExternalInput")
    wp18r = nc.dram_tensor("wp18r", [128, 2, H], F8, kind="ExternalInput")
    pp8c = nc.dram_tensor("pp8c", [128, 2, H], F8, kind="ExternalInput")
    w18 = nc.dram_tensor("w18", [128, 2, H], F8, kind="ExternalInput")
    w18r = nc.dram_tensor("w18r", [128, 2, H], F8, kind="ExternalInput")
    w28 = nc.dram_tensor("w28", [128, 2, H], F8, kind="ExternalInput")
    w28r = nc.dram_tensor("w28r", [128, 2, H], F8, kind="ExternalInput")
    qwh = nc.dram_tensor("qwh", [128, 2], F16, kind="ExternalInput")
    bcg = nc.dram_tensor("bcg", [S, H], F16, kind="ExternalInput")
    iota_at = nc.dram_tensor("iota_at", [128, S], F16, kind="ExternalInput")
    hs = nc.dram_tensor("hs", [C, 128, 2 * S], F32, kind="ExternalOutput")

    eq = mybir.AluOpType.is_equal
    mult = mybir.AluOpType.mult
    add = mybir.AluOpType.add
    Tanh = mybir.ActivationFunctionType.Tanh

    with tile.TileContext(nc) as tc:
        with (
            tc.tile_pool(name="consts", bufs=1) as pc,
            tc.tile_pool(name="work", bufs=4) as pwk,
            # PSUM banks: ph 1x2 + gate 2x2 + ga 2x1 = 8
            tc.tile_pool(name="pph", bufs=1, space="PSUM") as pph,
            tc.tile_pool(name="pgt", bufs=2, space="PSUM") as pgt,
            tc.tile_pool(name="pga", bufs=2, space="PSUM") as pga,
        ):
            # ---- constants ----
            def cload(t, shape, dt):
                nm = f"c_{t.name}"
                sb = pc.tile(shape, dt, name=nm, tag=nm)
                nc.sync.dma_start(out=sb, in_=t[:])
                return sb

            wp18_sb = cload(wp18, [128, 2, H], F8)
            wp18r_sb = cload(wp18r, [128, 2, H], F8)
            pp8c_sb = cload(pp8c, [128, 2, H], F8)
            w18_sb = cload(w18, [128, 2, H], F8)
            w18r_sb = cload(w18r, [128, 2, H], F8)
            w28_sb = cload(w28, [128, 2, H], F8)
            w28r_sb = cload(w28r, [128, 2, H], F8)
            qwh_sb = cload(qwh, [128, 2], F16)
            bcg_sb = cload(bcg, [S, H], F16)
            iota_at_sb = cload(iota_at, [128, S], F16)
            segc_sb = cload(segc, [128, C, KT], F32)
            rec_sb = cload(recip, [S, C], F32)

            # cross-iteration tile handles (3-stage software pipeline)
            T_xm, T_x16, T_as, T_at, T_ph8, T_g1, T_gt, T_ga, T_smt = (
                {}, {}, {}, {}, {}, {}, {}, {}, {}
            )
            T_gp, T_hs2 = {}, {}

            def emit_loads(c):
                np_ = min(2, C - c)
                xm_t = pwk.tile([128, 2, 5 * TC], F8, tag="xm")
                nc.sync.dma_start(
                    out=xm_t[:, :np_, :],
                    in_=xm8[c : c + np_].rearrange("c p m -> p c m"),
                )
                x16_t = pwk.tile([128, 2, KT * H], F16, tag="x16")
                nc.sync.dma_start(
                    out=x16_t[:, :np_, :],
                    in_=x16[c : c + np_].rearrange("c p m -> p c m"),
                )
                as_t = pwk.tile([S, 2, TC], F16, tag="as")
                nc.sync.dma_start(
                    out=as_t[:, :np_, :],
                    in_=a_s[c : c + np_].rearrange("c p m -> p c m"),
                )
                for j in range(np_):
                    T_xm[c + j] = xm_t[:, j, :]
                    T_x16[c + j] = x16_t[:, j, :]
                    T_as[c + j] = as_t[:, j, :]

            def dr(out, lhsT, rhs, start, stop):
                nc.tensor.matmul(out, lhsT, rhs, start=start, stop=stop,
                                 perf_mode=DR)

            emit_loads(0)
            if C > 2:
                emit_loads(2)
            for it in range(C + 2):
                c0 = it      # masks + ph(+tanh) + ss + g1
                c1 = it - 1  # gate(+tanh)
                c2 = it - 2  # alpha + h_sT + store
                if c0 % 2 == 0 and c0 + 4 < C:
                    emit_loads(c0 + 4)

                # ---- gate(c1) first: all deps are >= 1 iteration old, so
                # the PE can start immediately and ACT gets its gate tanh
                # early instead of bunching both tanhs late.
                if 0 <= c1 < C:
                    ph8_1 = T_ph8.pop(c1)
                    g11 = T_g1.pop(c1)
                    as1 = T_as.pop(c1)
                    gp = pgt.tile([128, 2, TC], F32, tag="gate")
                    for h in range(2):
                        dst = gp[:, h, :]
                        lo, hi = h * 128, (h + 1) * 128
                        dr(dst, w28_sb[:, :, lo:hi], ph8_1[:], True, False)
                        dr(dst, w28r_sb[:, :, lo:hi], ph8_1[:], False, False)
                        nc.tensor.matmul(dst, g11[:, h, :], as1,
                                         start=False, stop=True)
                    gt1 = pwk.tile([128, 2, TC], F16, tag="gt")
                    nc.scalar.activation(
                        out=gt1.rearrange("p j t -> p (j t)"),
                        in_=gp.rearrange("p j t -> p (j t)"),
                        func=Tanh, scale=0.5,
                    )
                    T_gt[c1] = gt1
                    T_gp[c1] = gp

                # ---- a_t masks for c0 (DVE) ----
                if c0 < C:
                    a_t = pwk.tile([128, KT, S], F16, tag="a_t")
                    for k in range(KT):
                        nc.vector.tensor_single_scalar(
                            out=a_t[:, k, :], in_=iota_at_sb,
                            scalar=segc_sb[:, c0, k : k + 1], op=eq,
                        )
                    T_at[c0] = a_t

                # ---- alpha(c2): PE contraction of gate with qw ----
                if c2 >= 0:
                    gt = T_gt.pop(c2)
                    gp2 = T_gp[c2]
                    alp = gp2[:, 0, 0:KT]
                    for kt in range(KT):
                        for h in range(2):
                            nc.tensor.matmul(
                                alp[:, kt : kt + 1],
                                gt[:, h, kt * 128 : (kt + 1) * 128],
                                qwh_sb[:, h : h + 1],
                                start=(h == 0), stop=(h == 1),
                            )

                # ---- ph(c0): fp8 DR with residual pairs + pos one-hot ----
                if c0 < C:
                    xm_c = T_xm[c0]
                    xmv = xm_c.rearrange("p (j t) -> p j t", j=5)
                    oh_rep = bass.AP(
                        tensor=xm_c.tensor, offset=xmv[:, 4, :].offset,
                        ap=[list(xmv.ap[0])] + [[0, 2], [1, TC]],
                    )
                    php = pph.tile([128, 2, TC], F32, tag="ph")
                    for h in range(2):
                        dst = php[:, h, :]
                        lo, hi = h * 128, (h + 1) * 128
                        dr(dst, wp18_sb[:, :, lo:hi], xmv[:, 0:2, :], True, False)
                        dr(dst, wp18_sb[:, :, lo:hi], xmv[:, 2:4, :], False, False)
                        dr(dst, wp18r_sb[:, :, lo:hi], xmv[:, 0:2, :], False, False)
                        dr(dst, pp8c_sb[:, :, lo:hi], oh_rep, False, True)
                    ph8 = pwk.tile([128, 2, TC], F8, tag="ph8")
                    nc.scalar.activation(
                        out=ph8.rearrange("p j t -> p (j t)"),
                        in_=php.rearrange("p j t -> p (j t)"),
                        func=Tanh,
                    )
                    T_ph8[c0] = ph8

                # ---- ss(c0) + g1(c0): whole v_mean pipeline in stage c0 so
                # gate(c0) next iteration has no same-iteration deps.
                if c0 < C:
                    x16_c = T_x16[c0]
                    a_t0 = T_at[c0]
                    ga = pga.tile([128, 2 * S + H], F32, tag="ga")
                    ss = ga[:, 0 : 2 * S]
                    for h in range(2):
                        lo, hi = h * 128, (h + 1) * 128
                        for k in range(KT):
                            nc.tensor.matmul(
                                ss[:, h * S : (h + 1) * S],
                                x16_c[:, k * H + lo : k * H + hi],
                                a_t0[:, k, :],
                                start=(k == 0), stop=(k == KT - 1),
                            )
                    smt = pwk.tile([128, 2 * S], F8, tag="smt")
                    nc.vector.tensor_copy(out=smt, in_=ss)
                    g1p = ga[0:S, 2 * S : 2 * S + H]
                    smt_pairs = smt.rearrange("p (j s) -> p j s", j=2)
                    dr(g1p, smt_pairs, w18_sb[:], True, False)
                    dr(g1p, smt_pairs, w18r_sb[:], False, True)
                    g1 = pwk.tile([S, 2, 128], F16, tag="g1")
                    nc.vector.scalar_tensor_tensor(
                        out=g1.rearrange("s j m -> s (j m)"), in0=g1p,
                        scalar=rec_sb[:, c0 : c0 + 1], in1=bcg_sb,
                        op0=mult, op1=add,
                    )
                    T_g1[c0] = g1

                # ---- finish alpha(c2), transposed h_s(c2) ----
                if c2 >= 0:
                    x16_2 = T_x16.pop(c2)
                    a_t2 = T_at.pop(c2)
                    gp2 = T_gp.pop(c2)
                    del T_xm[c2]
                    alp_sb = pwk.tile([128, KT], F32, tag="alp")
                    nc.vector.tensor_scalar(
                        out=alp_sb, in0=gp2[:, 0, 0:KT], scalar1=0.5,
                        scalar2=qbp, op0=mult, op1=add,
                    )
                    aat = pwk.tile([128, KT, S], F16, tag="aat")
                    for k in range(KT):
                        nc.vector.tensor_single_scalar(
                            out=aat[:, k, :], in_=a_t2[:, k, :],
                            scalar=alp_sb[:, k : k + 1], op=mult,
                        )
                    hsp = gp2[:, 1, TC - 2 * S : TC]
                    for h in range(2):
                        lo, hi = h * 128, (h + 1) * 128
                        for k in range(KT):
                            nc.tensor.matmul(
                                hsp[:, h * S : (h + 1) * S],
                                x16_2[:, k * H + lo : k * H + hi],
                                aat[:, k, :],
                                start=(k == 0), stop=(k == KT - 1),
                            )
                    if c2 % 2 == 0:
                        T_hs2[c2 // 2] = pwk.tile(
                            [128, 2, 2 * S], F32, tag="hs2", name="hs2"
                        )
                    hs2 = T_hs2[c2 // 2]
                    nc.vector.tensor_copy(out=hs2[:, c2 % 2, :], in_=hsp)
                    if c2 % 2 == 1 or c2 == C - 1:
                        np_ = c2 % 2 + 1
                        lo_c = c2 - np_ + 1
                        nc.gpsimd.dma_start(
                            out=hs[lo_c : c2 + 1].rearrange("c p m -> p c m"),
                            in_=hs2[:, :np_, :],
                        )
                        del T_hs2[c2 // 2]

    _split_multi_waits(nc)
    return nc


_CACHE = {}


def kernel(hidden, pos_table, W_pos, b_pos, W1, b1, W2, b2, qw, qb,
           seq_len, reverse_pos):
    seq_len_np = np.asarray(seq_len)
    C, bins = _plan(seq_len_np)
    weights, qbp = _pack_weights(pos_table, W_pos, b_pos, W1, b1, W2, b2, qw, qb)
    xm8, x16, a_s, seg_col, recip, unpack_idx = _pack_inputs(
        hidden, reverse_pos, seq_len_np, C, bins
    )

    key = (C, qbp)
    if key not in _CACHE:
        _CACHE[key] = _build_bass(C, qbp)
    nc = _CACHE[key]

    in_maps = []
    for core in range(N_CORES):
        m = dict(
            xm8=xm8[core], x16=x16[core], a_s=a_s[core],
            segc=seg_col[core], recip=recip[core],
        )
        m.update(weights)
        in_maps.append(m)

    import time as _time

    t0 = _time.perf_counter()
    res = run_bass_kernel_spmd(nc, in_maps, core_ids=list(range(N_CORES)))
    kernel._last_run_s = _time.perf_counter() - t0
    hs_all = np.stack([res.results[i]["hs"] for i in range(N_CORES)])

    out_core, out_chunk, out_local = unpack_idx
    B = len(out_core)
    tmp = hs_all[out_core, out_chunk]          # [B, 128, 2S]
    tmp = tmp.reshape(B, 128, 2, S)
    res_b = tmp[np.arange(B), :, :, out_local]  # [B, 128, 2]
    return np.ascontiguousarray(
        res_b.transpose(0, 2, 1).reshape(B, H).astype(np.float32)
    )
